# revision 1
# baseline (speedup 1.0000x reference)
"""Trainium2 Bass kernel for nn_CWAUCHLoss (pairwise AUC hinge + class-weighted CE).

Math: with s = sigmoid(output[:, 0]), lab = labels[:, 0], the O(B^2) pairwise
penalty collapses algebraically (LAMB == 2):

  sum_{i in N, j in P} (1 - (s_j - s_i))^2
    = n_pos * sum_N s^2 + 2 * (sum_N s)(sum_P (1-s)) + n_neg * sum_P (1-s)^2

so the whole loss reduces to 7 masked sums over B elements:
  r0 = sum lab          r1 = sum s       r2 = sum s^2
  r3 = sum lab*s        r4 = sum lab*s^2
  q1 = sum ln(1+e^-s)   q2 = sum lab*ln(1+e^-s)
using ln(t) = -ln(1+e^-s) and ln(1-t) = ln(t) - s for t = sigmoid(s).

On-chip (one NeuronCore, batch spread over 128 partitions x 64 lanes):
  - sigmoid is computed as 1/(1+exp(-x)) so ALL transcendentals (Exp, Ln)
    live in one ACT table set (a Sigmoid->Ln sequence would pay a ~2.7us
    mid-kernel table switch); a dummy Exp prewarms the table during the
    input DMA, and a post-compile pass retargets the auto-inserted table
    load to the combined natural_log_exp_and_others set.
  - per-partition sums land in a [128, 8] stats tile via activation/STT
    accumulators; a matmul against the tile's own 1/128 constant column
    reduces across partitions; a second matmul against a constant 8x12
    matrix forms every linear combination; a 5-product bilinear form +
    grouped reduce yields [numerator, fpcls].
  - the penalty denominator 2*r0*(B-r0) depends only on the label count,
    which lands ~1.3us before the ln chain finishes, so 1/den is computed
    on a DVE side chain (tiny r0-only matmul -> affine -> reciprocal)
    entirely inside that idle window, off the critical tail.
  - raw Bass (nc.Block) with per-engine counter semaphores: TRN2 engines
    are deep-pipelined with no scoreboard, so every RAW dependency (same-
    engine included) is sequenced through semaphores; skipping Tile's
    entry/exit barriers saves ~0.4us on a ~8.4us kernel.
"""

import numpy as np

B = 8192
P = 128
N = B // P  # 64 elements per partition

_nc_cache = None


def _wmat() -> np.ndarray:
    # Rows index the partition-reduced stats rc = [r0,r1,r2,r3,r4,1,q1,q2]/128.
    # Cols 0-5 build vector A, cols 6-11 build vector B; elementwise A*B then
    # group-sum by 3 gives [penalty numerator, fpcls] (the denominator is
    # handled by the r0-only side chain).
    W = np.zeros((8, 12), dtype=np.float64)
    Bf = float(B)
    W[0, 0] = 1.0                                   # A0 = r0
    W[1, 1] = 2.0
    W[3, 1] = -2.0                                  # A1 = 2(r1-r3)
    W[5, 2] = Bf
    W[0, 2] = -1.0                                  # A2 = B-r0
    W[5, 3] = 1.0 / Bf                              # A3 = 1/B
    W[0, 4] = 1.0 / (Bf * Bf)                       # A4 = r0/B^2
    W[2, 6] = 1.0
    W[4, 6] = -1.0                                  # B0 = r2-r4
    W[0, 7] = 1.0
    W[3, 7] = -1.0                                  # B1 = r0-r3
    W[0, 8] = 1.0
    W[3, 8] = -2.0
    W[4, 8] = 1.0                                   # B2 = r0-2r3+r4
    W[7, 9] = 1.0                                   # B3 = q2
    W[6, 10] = 1.0
    W[1, 10] = 1.0
    W[7, 10] = -2.0
    W[3, 10] = -1.0                                 # B4 = q1+r1-2q2-r3
    # rc carries true_sums/128 (the reduce matmul weights by the 1/128 const
    # column), so scale every coefficient by 128 to compensate.
    return np.ascontiguousarray(W * P, dtype=np.float32)


def build_nc():
    from contextlib import ExitStack

    import concourse.bacc as bacc
    import concourse.mybir as mybir

    f32 = mybir.dt.float32
    AF = mybir.ActivationFunctionType
    ALU = mybir.AluOpType
    AX = mybir.AxisListType

    nc = bacc.Bacc(None, target_bir_lowering=False, debug=False)
    x_d = nc.dram_tensor("output", [B, 2], f32, kind="ExternalInput")
    l_d = nc.dram_tensor("labels", [B, 1], f32, kind="ExternalInput")
    w_d = nc.dram_tensor("wmat", [8, 12], f32, kind="ExternalInput")
    o_d = nc.dram_tensor("out", [1, 2], f32, kind="ExternalOutput")

    with ExitStack() as ctx:
        e = ctx.enter_context
        xt = e(nc.sbuf_tensor([P, N, 2], f32))
        lt = e(nc.sbuf_tensor([P, N], f32))
        wt = e(nc.sbuf_tensor([8, 12], f32))
        e1 = e(nc.sbuf_tensor([P, N], f32))
        p1 = e(nc.sbuf_tensor([P, N], f32))
        s = e(nc.sbuf_tensor([P, N], f32))
        ls = e(nc.sbuf_tensor([P, N], f32))
        e2 = e(nc.sbuf_tensor([P, N], f32))
        lnw = e(nc.sbuf_tensor([P, N], f32))
        scr2 = e(nc.sbuf_tensor([P, N], f32))
        scr4 = e(nc.sbuf_tensor([P, N], f32))
        scrq = e(nc.sbuf_tensor([P, N], f32))
        ST = e(nc.sbuf_tensor([P, 8], f32))
        warm = e(nc.sbuf_tensor([1, 2], f32))
        bias01 = e(nc.sbuf_tensor([P, 2], f32))
        rcs = e(nc.sbuf_tensor([8, 1], f32))
        LCs = e(nc.sbuf_tensor([1, 12], f32))
        PPt = e(nc.sbuf_tensor([1, 6], f32))
        Ft = e(nc.sbuf_tensor([1, 4], f32))
        r0s = e(nc.sbuf_tensor([1, 4], f32))  # [r0/128, t1, den, invden]
        G = e(nc.sbuf_tensor([1, 2], f32))
        psA = e(nc.psum_tensor([8, 1], f32))
        psB = e(nc.psum_tensor([1, 12], f32))
        psR = e(nc.psum_tensor([1, 1], f32))
        d_x = e(nc.semaphore("d_x"))
        d_l = e(nc.semaphore("d_l"))
        d_w = e(nc.semaphore("d_w"))
        d_o = e(nc.semaphore("d_o"))
        ACTc = e(nc.semaphore("ACTc"))
        DVEc = e(nc.semaphore("DVEc"))
        PEc = e(nc.semaphore("PEc"))
        block = e(nc.Block())

        @block.sync
        def _(sync):
            # x first: it gates the whole compute chain (HWDGE descriptor
            # generation is a shared serial unit, ~625ns per dma_start).
            sync.dma_start(
                xt[:], x_d.ap().rearrange("(p n) c -> p n c", p=P)
            ).then_inc(d_x, 16)
            sync.dma_start(
                lt[:], l_d.ap().rearrange("(p n) c -> p (n c)", p=P)
            ).then_inc(d_l, 16)
            sync.wait_ge(DVEc, 22)  # G written
            sync.dma_start(o_d.ap(), G[:]).then_inc(d_o, 16)
            sync.wait_ge(d_o, 16)

        @block.gpsimd
        def _(gpsimd):
            # wmat is needed late (second matmul); SWDGE generation on the
            # otherwise-idle Pool engine runs parallel to the HWDGE unit.
            gpsimd.dma_start(wt[:], w_d.ap()).then_inc(d_w, 16)

        @block.scalar
        def _(scalar):
            scalar.wait_ge(DVEc, 3)  # bias01 + warm tile memsets
            # prewarm: pulls the exp/ln table set during the input DMA
            scalar.activation(
                warm[:], warm[:], AF.Exp, bias=bias01[0:1, 0:1]
            ).then_inc(ACTc, 1)  # 1
            scalar.wait_ge(d_x, 16)
            scalar.activation(
                e1[:], xt[:, :, 0], AF.Exp, scale=-1.0, bias=bias01[:, 0:1]
            ).then_inc(ACTc, 1)  # 2
            scalar.activation(
                e2[:], s[:], AF.Exp, scale=-1.0, bias=bias01[:, 0:1]
            ).then_inc(ACTc, 1)._wait_ge(DVEc, 7)  # 3
            # ln(1+e2): the +1 rides the Ln op's bias input; accum -> q1
            scalar.activation(
                lnw[:], e2[:], AF.Ln, bias=bias01[:, 1:2],
                accum_out=ST[:, 6:7],
            ).then_inc(ACTc, 1)._wait_ge(ACTc, 3)  # 4

        @block.vector
        def _(vector):
            # dep-free preamble memsets (compute path => inc-by-1 legal;
            # gpsimd memsets with inc-1 sems crash the device)
            vector.memset(bias01[:, 0:1], 0.0).then_inc(DVEc, 1)   # 1
            vector.memset(bias01[:, 1:2], 1.0).then_inc(DVEc, 1)   # 2
            vector.memset(warm[:], 1.0).then_inc(DVEc, 1)          # 3
            vector.memset(Ft[:, 2:3], 0.0).then_inc(DVEc, 1)       # 4
            vector.memset(ST[:, 5:6], 1.0 / P).then_inc(DVEc, 1)   # 5
            # s = sigmoid(x0) = 1/(1+e1); reciprocal on DVE is IEEE-exact
            vector.tensor_scalar_add(
                p1[:], e1[:], 1.0
            ).then_inc(DVEc, 1)._wait_ge(ACTc, 2)  # 6
            vector.reciprocal(s[:], p1[:]).then_inc(DVEc, 1)._wait_ge(DVEc, 6)  # 7
            # per-partition stats (fill DVE idle time under the ACT chain);
            # r0 first: it feeds the invden side chain below
            vector.wait_ge(d_l, 16)
            vector.tensor_reduce(
                ST[:, 0:1], lt[:], axis=AX.X, op=ALU.add
            ).then_inc(DVEc, 1)  # 8
            vector.tensor_reduce(
                ST[:, 1:2], s[:], axis=AX.X, op=ALU.add
            ).then_inc(DVEc, 1)._wait_ge(DVEc, 7)  # 9
            vector.scalar_tensor_tensor(
                out=ls[:], in0=lt[:], scalar=1.0, in1=s[:],
                op0=ALU.mult, op1=ALU.mult, accum_out=ST[:, 3:4],
            ).then_inc(DVEc, 1)  # 10
            vector.scalar_tensor_tensor(
                out=scr2[:], in0=s[:], scalar=1.0, in1=s[:],
                op0=ALU.mult, op1=ALU.mult, accum_out=ST[:, 2:3],
            ).then_inc(DVEc, 1)  # 11
            vector.scalar_tensor_tensor(
                out=scr4[:], in0=ls[:], scalar=1.0, in1=ls[:],
                op0=ALU.mult, op1=ALU.mult, accum_out=ST[:, 4:5],
            ).then_inc(DVEc, 1)._wait_ge(DVEc, 10)  # 12
            # invden side chain: den = 2*r0*(B-r0) depends only on r0, so
            # 1/den is ready long before q2 and leaves the critical tail.
            # rc0 = r0/128, so den = rc0*(2*128*B - 2*128^2*rc0); all
            # coefficients are powers of two (exact in f32).
            vector.tensor_copy(r0s[0:1, 0:1], psR[:]).then_inc(DVEc, 1)._wait_ge(PEc, 1)  # 13
            vector.tensor_scalar(
                out=r0s[0:1, 1:2], in0=r0s[0:1, 0:1],
                scalar1=-2.0 * 128.0 * 128.0, scalar2=2.0 * 128.0 * 8192.0,
                op0=ALU.mult, op1=ALU.add,
            ).then_inc(DVEc, 1)._wait_ge(DVEc, 13)  # 14
            vector.tensor_tensor(
                r0s[0:1, 2:3], r0s[0:1, 0:1], r0s[0:1, 1:2], op=ALU.mult
            ).then_inc(DVEc, 1)._wait_ge(DVEc, 14)  # 15
            vector.reciprocal(
                r0s[0:1, 3:4], r0s[0:1, 2:3]
            ).then_inc(DVEc, 1)._wait_ge(DVEc, 15)  # 16
            # last stat: q2 (gated by the ln chain)
            vector.scalar_tensor_tensor(
                out=scrq[:], in0=lt[:], scalar=1.0, in1=lnw[:],
                op0=ALU.mult, op1=ALU.mult, accum_out=ST[:, 7:8],
            ).then_inc(DVEc, 1)._wait_ge(ACTc, 4)  # 17
            # tail: PSUM staging copies, bilinear products, grouped sums,
            # then G = [num*invden + fpcls, num*invden] = [cls, penalty]
            vector.tensor_copy(rcs[:], psA[:]).then_inc(DVEc, 1)._wait_ge(PEc, 2)  # 18
            vector.tensor_copy(LCs[:], psB[:]).then_inc(DVEc, 1)._wait_ge(PEc, 3)  # 19
            vector.tensor_tensor(
                PPt[:], LCs[0:1, 0:6], LCs[0:1, 6:12], op=ALU.mult
            ).then_inc(DVEc, 1)._wait_ge(DVEc, 19)  # 20
            vector.tensor_reduce(
                Ft[:, 0:2],
                PPt[:].rearrange("p (g k) -> p g k", k=3),
                axis=AX.X,
                op=ALU.add,
            ).then_inc(DVEc, 1)._wait_ge(DVEc, 20)  # 21
            vector.scalar_tensor_tensor(
                out=G[:],
                in0=Ft[:, 0:1].broadcast_to([1, 2]),
                scalar=r0s[0:1, 3:4],
                in1=Ft[:, 1:3],
                op0=ALU.mult,
                op1=ALU.add,
            ).then_inc(DVEc, 1)._wait_ge(DVEc, 21)  # 22

        @block.tensor
        def _(tensor):
            # r0-only reduce for the invden side chain (r0 lands early)
            tensor.matmul(
                psR[:], ST[:, 0:1], ST[:, 5:6]
            ).then_inc(PEc, 1)._wait_ge(DVEc, 8)
            tensor.wait_ge(ACTc, 4)   # lnw accum (q1)
            # cross-partition reduce: rc = ST^T @ (1/128 column)
            tensor.matmul(
                psA[:], ST[:, 0:8], ST[:, 5:6]
            ).then_inc(PEc, 1)._wait_ge(DVEc, 17)
            tensor.wait_ge(d_w, 16)   # wt
            # all linear combos: LC = rc^T @ W
            tensor.matmul(
                psB[:], rcs[:], wt[:]
            ).then_inc(PEc, 1)._wait_ge(DVEc, 18)

    nc.compile()

    # Table-load surgery: the greedy chooser assigns set 0 (exp_and_others)
    # to the Exp ops and then pays a second mid-chain ~1.3us load of set 5
    # (natural_log) before Ln.  Set 6 (natural_log_exp_and_others) contains
    # BOTH, so retarget the first load and drop the rest (they carry no
    # semaphore waits/updates).
    _COMBINED_EXP_LN_SET = 6
    for blk in nc.main_func.blocks:
        loads = [
            i for i in blk.instructions
            if isinstance(i, mybir.InstLoadActFuncSet)
        ]
        if not loads:
            continue
        assert all(not i.has_wait() and not i.has_update() for i in loads)
        loads[0].act_func_set_id = _COMBINED_EXP_LN_SET
        drop = {id(i) for i in loads[1:]}
        kept = [i for i in blk.instructions if id(i) not in drop]
        del blk.instructions[:]
        blk.instructions.extend(kept)

    # Drop Bass.__init__'s unconditional const-AP memsets (f32 0/1, bf16 1,
    # u8 127): nothing in this kernel reads them (biases come from bias01).
    import json as _json

    for blk in nc.main_func.blocks:
        kept = []
        for i in blk.instructions:
            if isinstance(i, mybir.InstMemset) and not i.has_wait() and not i.has_update():
                j = _json.loads(mybir.instruction_to_pretty_json_string(i))
                memref = j.get("outs", [{}])[0].get("memref", "")
                if isinstance(memref, str) and memref.startswith("const-"):
                    continue
            kept.append(i)
        if len(kept) != len(blk.instructions):
            del blk.instructions[:]
            blk.instructions.extend(kept)
    return nc


def _in_map(output: np.ndarray, labels: np.ndarray) -> dict:
    return {
        "output": np.ascontiguousarray(output, dtype=np.float32),
        "labels": np.ascontiguousarray(labels, dtype=np.float32),
        "wmat": _wmat(),
    }


def kernel(output: np.ndarray, labels: np.ndarray) -> np.ndarray:
    global _nc_cache
    from concourse.bass_utils import run_bass_kernel_spmd

    if _nc_cache is None:
        _nc_cache = build_nc()
    res = run_bass_kernel_spmd(_nc_cache, [_in_map(output, labels)], core_ids=[0])
    g = res.results[0]["out"]
    return np.asarray(g, dtype=np.float32).reshape(2).copy()



# revision 2
# speedup vs baseline: 1.0396x; 1.0396x over previous
"""Trainium2 Bass kernel for nn_CWAUCHLoss — v2 (latency-restructured).

Math: with s = sigmoid(x0), lab = labels, LAMB == 2, the O(B^2) pairwise
penalty collapses to sums r0..r4 (see baseline).  v2 additionally removes
every dependent transcendental:

  - softplus term ln(1+e^-s), s in (0,1), is replaced by its degree-2
    Chebyshev fit c0 + c1 s + c2 s^2 (max err 5.2e-4); the coefficients
    fold into the W matrix, so no per-element op is needed at all.
  - s^2 is replaced by a + b*sigmoid(c*x + d) (fit of sigmoid(x)^2; the
    residual largely cancels between r2 and r4).  This makes the second
    ACT op independent of the first, so the ACT engine streams
    sigmoid(x), sigmoid(c*x+d) back-to-back with no RAW gap.

  End-to-end model error vs the f64 reference: ~4e-5 relative.

Stats (true sums): r0=sum lab, r1=sum s, T2=sum g2, r3=sum lab*s,
T4=sum lab*g2 where g2 = sigmoid(c*x+d); r2 = a*B + b*T2 and
r4 = a*r0 + b*T4 are substituted inside W.

Pipeline (one core):
  DMA in: one [128,128] f32 tile (x0 | lab), 512B/partition lines.
  ACT: s = Sigmoid(x); g2 = Sigmoid(c*x+d) with accum -> T2 column.
  DVE: reduce(lab), stt lab*s (acc r3), reduce(s), stt lab*g2 (acc T4);
       side chain den' = (rc0-B/128)*rc0 from the r0-only matmul psR,
       reciprocal -> inv' (the -1/32768 scale folds into W's A-columns).
  PE:  psR (r0 only, early), psA[6,1] = ST^T @ (1/128 col),
       psB[1,12] = rcs^T @ W  (A lanes 0:5, B lanes 6:11).
  DVE tail: F-num STT  acc -> pen   = sum (A*inv')∘B over lanes 0:3
            F-fp  TTR  acc -> cls   = pen + sum A∘B over lanes 3:5
       so Fcomb[0, 0:2] = [cls, pen] and DMAs out directly.

Post-compile surgery: drop Bass's unconditional const-AP memsets, and
move the input DMACopy ahead of SP's entry-barrier wait so HWDGE
descriptor generation starts ~250ns earlier.
"""

import numpy as np

B = 8192
P = 128
N = B // P  # 64

# sigmoid(x)^2 ~= FA + FB*sigmoid(FC*x + FD)  (density-weighted LSQ fit)
FA = -0.02057171589895669
FB = 0.9629407956703709
FC = 1.2584236137434404
FD = -0.9461196661369293
# ln(1+e^-s) ~= C0 + C1*s + C2*s^2 on s in (0,1)
C0 = 0.6927389880843673
C1 = -0.4956064759975386
C2 = 0.11664985996148536
# pen = num/den_true; den_true = -2*128^2 * den' with den'=(rc0-B/128)*rc0
KPEN = -1.0 / 32768.0

_nc_cache = None

# TensorTensorReduce compiles but dies at runtime on the PJRT/axon path;
# use the two-op STT tail instead.
USE_TTR = False
# The out-DMA still carries its completion-semaphore increment (walrus
# requires one), but nothing waits on it: the runtime's queue tracking
# covers output delivery, saving the explicit wait + drain tail.
WAIT_DO = False
SP_DMA_SURGERY = True
FP16 = False


def _wmat() -> np.ndarray:
    # Rows: true-sum stats [r0, r1, T2, r3, T4, 1].  Cols 0-5 = A lanes,
    # 6-11 = B lanes of psB.  psB = rcs^T @ W with rcs = true/128, so
    # multiply everything by 128 at the end.
    Bf = float(B)
    W = np.zeros((6, 12), dtype=np.float64)
    # A0' = KPEN * r0
    W[0, 0] = KPEN
    # A1' = KPEN * 2(r1 - r3)
    W[1, 1] = 2.0 * KPEN
    W[3, 1] = -2.0 * KPEN
    # A2' = KPEN * (B - r0)
    W[5, 2] = Bf * KPEN
    W[0, 2] = -KPEN
    # A3 = 1/B
    W[5, 3] = 1.0 / Bf
    # A4 = r0/B^2
    W[0, 4] = 1.0 / (Bf * Bf)
    # B0 = r2 - r4 = a*B - a*r0 + b*T2 - b*T4
    W[5, 6] = FA * Bf
    W[0, 6] = -FA
    W[2, 6] = FB
    W[4, 6] = -FB
    # B1 = r0 - r3
    W[0, 7] = 1.0
    W[3, 7] = -1.0
    # B2 = r0 - 2 r3 + r4 = (1+a) r0 - 2 r3 + b*T4
    W[0, 8] = 1.0 + FA
    W[3, 8] = -2.0
    W[4, 8] = FB
    # B3 = q2 = c0 r0 + c1 r3 + c2 r4 = (c0 + c2 a) r0 + c1 r3 + c2 b T4
    W[0, 9] = C0 + C2 * FA
    W[3, 9] = C1
    W[4, 9] = C2 * FB
    # B4 = q1 + r1 - 2 q2 - r3
    #    = (c0 B + c2 a B) + (c1+1) r1 + c2 b T2
    #      + (-2 c0 - 2 c2 a) r0 - (2 c1 + 1) r3 - 2 c2 b T4
    W[5, 10] = C0 * Bf + C2 * FA * Bf
    W[1, 10] = C1 + 1.0
    W[2, 10] = C2 * FB
    W[0, 10] = -2.0 * C0 - 2.0 * C2 * FA
    W[3, 10] = -(2.0 * C1 + 1.0)
    W[4, 10] = -2.0 * C2 * FB
    return np.ascontiguousarray(W * P, dtype=np.float32)


def build_nc():
    from contextlib import ExitStack

    import concourse.bacc as bacc
    import concourse.mybir as mybir

    f32 = mybir.dt.float32
    f16 = mybir.dt.float16 if FP16 else mybir.dt.float32
    AF = mybir.ActivationFunctionType
    ALU = mybir.AluOpType
    AX = mybir.AxisListType

    nc = bacc.Bacc(None, target_bir_lowering=False, debug=False)
    x_d = nc.dram_tensor("xcat", [P, 2 * N], f16, kind="ExternalInput")
    w_d = nc.dram_tensor("wmat", [6, 12], f32, kind="ExternalInput")
    o_d = nc.dram_tensor("out", [1, 2], f32, kind="ExternalOutput")

    with ExitStack() as ctx:
        e = ctx.enter_context
        xt = e(nc.sbuf_tensor([P, 2 * N], f16))   # cols 0:64 x0, 64:128 lab
        s = e(nc.sbuf_tensor([P, N], f16))
        g2 = e(nc.sbuf_tensor([P, N], f16))
        scr1 = e(nc.sbuf_tensor([P, N], f16))
        scr2 = e(nc.sbuf_tensor([P, N], f16))
        ST = e(nc.sbuf_tensor([P, 6], f32))
        bias0 = e(nc.sbuf_tensor([P, 1], f32))
        biasd = e(nc.sbuf_tensor([P, 1], f32))
        wt = e(nc.sbuf_tensor([6, 12], f32))
        rcs = e(nc.sbuf_tensor([6, 1], f32))
        rb = e(nc.sbuf_tensor([1, 1], f32))
        LCsb = e(nc.sbuf_tensor([1, 12], f32))
        t1s = e(nc.sbuf_tensor([1, 1], f32))
        fps = e(nc.sbuf_tensor([1, 1], f32))
        invs = e(nc.sbuf_tensor([1, 1], f32))
        Fcomb = e(nc.sbuf_tensor([1, 2], f32))    # [cls, pen]
        scrF = e(nc.sbuf_tensor([1, 3], f32))
        scrT = e(nc.sbuf_tensor([1, 2], f32))
        psR = e(nc.psum_tensor([1, 1], f32))
        psA = e(nc.psum_tensor([6, 1], f32))
        psB = e(nc.psum_tensor([1, 12], f32))
        d_x = e(nc.semaphore("d_x"))
        d_w = e(nc.semaphore("d_w"))
        d_o = e(nc.semaphore("d_o"))
        ACTc = e(nc.semaphore("ACTc"))
        DVEc = e(nc.semaphore("DVEc"))
        PEc = e(nc.semaphore("PEc"))
        block = e(nc.Block())

        lt = xt[:, N : 2 * N]

        @block.sync
        def _(sync):
            sync.dma_start(xt[:], x_d.ap()).then_inc(d_x, 16)
            sync.wait_ge(DVEc, 14 if USE_TTR else 15)  # Fcomb complete
            sync.dma_start(o_d.ap(), Fcomb[0:1, 0:2]).then_inc(d_o, 16)
            if WAIT_DO:
                sync.wait_ge(d_o, 16)

        @block.gpsimd
        def _(gpsimd):
            gpsimd.dma_start(wt[:], w_d.ap()).then_inc(d_w, 16)

        @block.scalar
        def _(scalar):
            # s = sigmoid(x); g2 = sigmoid(FC*x + FD): independent, so the
            # ACT engine streams them back-to-back with no RAW gap.
            scalar.wait_ge(DVEc, 2)  # bias memsets
            scalar.activation(
                s[:], xt[:, 0:N], AF.Sigmoid, bias=bias0[:, 0:1]
            ).then_inc(ACTc, 1)._wait_ge(d_x, 16)
            scalar.activation(
                g2[:], xt[:, 0:N], AF.Sigmoid, bias=biasd[:, 0:1],
                scale=FC, accum_out=ST[:, 2:3],
            ).then_inc(ACTc, 1)

        @block.vector
        def _(vector):
            vector.memset(bias0[:, 0:1], 0.0).then_inc(DVEc, 1)       # 1
            vector.memset(biasd[:, 0:1], FD).then_inc(DVEc, 1)        # 2
            vector.memset(ST[:, 5:6], 1.0 / P).then_inc(DVEc, 1)      # 3
            vector.tensor_reduce(
                ST[:, 0:1], lt, axis=AX.X, op=ALU.add
            ).then_inc(DVEc, 1)._wait_ge(d_x, 16)                     # 4
            vector.scalar_tensor_tensor(
                out=scr1[:], in0=lt, scalar=1.0, in1=s[:],
                op0=ALU.mult, op1=ALU.mult, accum_out=ST[:, 3:4],
            ).then_inc(DVEc, 1)._wait_ge(ACTc, 1)                     # 5
            vector.tensor_reduce(
                ST[:, 1:2], s[:], axis=AX.X, op=ALU.add
            ).then_inc(DVEc, 1)                                       # 6
            vector.scalar_tensor_tensor(
                out=scr2[:], in0=lt, scalar=1.0, in1=g2[:],
                op0=ALU.mult, op1=ALU.mult, accum_out=ST[:, 4:5],
            ).then_inc(DVEc, 1)._wait_ge(ACTc, 2)                     # 7
            # side chain: rc0 copy, den' = (rc0 - B/128)*rc0, inv' = 1/den'
            # (runs entirely inside the psB-wait window, off the tail path)
            vector.tensor_copy(rb[0:1, 0:1], psR[:]).then_inc(DVEc, 1)._wait_ge(PEc, 1)  # 8
            vector.tensor_copy(rcs[:], psA[:]).then_inc(DVEc, 1)._wait_ge(PEc, 2)  # 9
            vector.scalar_tensor_tensor(
                out=t1s[0:1, 0:1], in0=rb[0:1, 0:1], scalar=-float(B) / P,
                op0=ALU.add, in1=rb[0:1, 0:1], op1=ALU.mult,
            ).then_inc(DVEc, 1)._wait_ge(DVEc, 8)                     # 10
            vector.reciprocal(
                invs[0:1, 0:1], t1s[0:1, 0:1]
            ).then_inc(DVEc, 1)._wait_ge(DVEc, 10)                    # 11
            # stage psB lanes in SBUF (DVE ops may read only one PSUM input)
            vector.tensor_copy(LCsb[0:1, 0:12], psB[0:1, 0:12]).then_inc(DVEc, 1)._wait_ge(PEc, 3)  # 12
            # pen = sum_j (A'_j * inv') * B_j over lanes 0:3  (A' has the
            # -1/32768 fold), accumulated into Fcomb lane 1
            vector.scalar_tensor_tensor(
                out=scrF[0:1, 0:3], in0=LCsb[0:1, 0:3],
                scalar=invs[0:1, 0:1], op0=ALU.mult,
                in1=LCsb[0:1, 6:9], op1=ALU.mult,
                accum_out=Fcomb[0:1, 1:2],
            ).then_inc(DVEc, 1)._wait_ge(DVEc, 12)                    # 13
            # cls = pen + sum_j A_j*B_j over lanes 3:5 into Fcomb lane 0
            if USE_TTR:
                vector.tensor_tensor_reduce(
                    out=scrT[0:1, 0:2], in0=LCsb[0:1, 3:5], in1=LCsb[0:1, 9:11],
                    scale=1.0, scalar=Fcomb[0:1, 1:2],
                    op0=ALU.mult, op1=ALU.add,
                    accum_out=Fcomb[0:1, 0:1],
                ).then_inc(DVEc, 1)._wait_ge(DVEc, 13)                # 14
            else:
                # fallback: fp into scratch accum (independent of F-num —
                # both only need LCsb, so they stream back-to-back), then
                # cls = pen + fp
                vector.scalar_tensor_tensor(
                    out=scrT[0:1, 0:2], in0=LCsb[0:1, 3:5], scalar=1.0,
                    op0=ALU.mult, in1=LCsb[0:1, 9:11], op1=ALU.mult,
                    accum_out=fps[0:1, 0:1],
                ).then_inc(DVEc, 1)                                   # 14a
                vector.tensor_tensor(
                    Fcomb[0:1, 0:1], fps[0:1, 0:1], Fcomb[0:1, 1:2],
                    op=ALU.add,
                ).then_inc(DVEc, 1)._wait_ge(DVEc, 14)                # 14b

        @block.tensor
        def _(tensor):
            tensor.matmul(
                psR[:], ST[:, 0:1], ST[:, 5:6]
            ).then_inc(PEc, 1)._wait_ge(DVEc, 4)
            tensor.wait_ge(ACTc, 2)
            tensor.matmul(
                psA[:], ST[:, 0:6], ST[:, 5:6]
            ).then_inc(PEc, 1)._wait_ge(DVEc, 7)
            tensor.wait_ge(d_w, 16)
            tensor.matmul(
                psB[:], rcs[:], wt[:]
            ).then_inc(PEc, 1)._wait_ge(DVEc, 9)

    nc.compile()

    import json as _json

    # Drop Bass.__init__'s unconditional const-AP memsets (biases are
    # explicit APs here; nothing reads the const APs).
    for blk in nc.main_func.blocks:
        kept = []
        for i in blk.instructions:
            if isinstance(i, mybir.InstMemset) and not i.has_wait() and not i.has_update():
                j = _json.loads(mybir.instruction_to_pretty_json_string(i))
                memref = j.get("outs", [{}])[0].get("memref", "")
                if isinstance(memref, str) and memref.startswith("const-"):
                    continue
            kept.append(i)
        if len(kept) != len(blk.instructions):
            del blk.instructions[:]
            blk.instructions.extend(kept)

    # Move the input DMACopy ahead of SP's entry-barrier EventSemaphore so
    # HWDGE descriptor generation overlaps the barrier rendezvous.  Keep it
    # after SP's entry Drain (the drain resets DMA semaphore state).
    if not SP_DMA_SURGERY:
        return nc
    EngineType = mybir.EngineType
    main_blk = nc.main_func.blocks[0]
    sp_sub = None
    for blk in nc.main_func.blocks[1:]:
        if any(i.engine == EngineType.SP and isinstance(i, mybir.InstDMACopy)
               for i in blk.instructions):
            sp_sub = blk
            break
    assert sp_sub is not None
    in_dma = next(
        i for i in sp_sub.instructions
        if isinstance(i, mybir.InstDMACopy) and not i.has_wait()
    )
    rest = [i for i in sp_sub.instructions if i is not in_dma]
    del sp_sub.instructions[:]
    sp_sub.instructions.extend(rest)
    # insert right after SP's Drain, before SP's barrier EventSemaphore
    idx = None
    for k, i in enumerate(main_blk.instructions):
        if isinstance(i, mybir.InstDrain) and i.engine == EngineType.SP:
            idx = k + 1
            break
    assert idx is not None
    main_blk.instructions.insert(idx, in_dma)
    return nc


def _in_map(output: np.ndarray, labels: np.ndarray) -> dict:
    x0 = np.ascontiguousarray(output[:, 0], dtype=np.float32).reshape(P, N)
    lab = np.ascontiguousarray(labels[:, 0], dtype=np.float32).reshape(P, N)
    xcat = np.concatenate([x0, lab], axis=1)
    return {
        "xcat": np.ascontiguousarray(
            xcat, dtype=np.float16 if FP16 else np.float32
        ),
        "wmat": _wmat(),
    }


def kernel(output: np.ndarray, labels: np.ndarray) -> np.ndarray:
    global _nc_cache
    from concourse.bass_utils import run_bass_kernel_spmd

    if _nc_cache is None:
        _nc_cache = build_nc()
    res = run_bass_kernel_spmd(_nc_cache, [_in_map(output, labels)], core_ids=[0])
    g = res.results[0]["out"]
    return np.asarray(g, dtype=np.float32).reshape(2).copy()
